# revision 5
# baseline (speedup 1.0000x reference)
"""Bass/Trainium2 kernel for nn_BranchedPolicyNetwork.

Computes out = tanh(features @ Wr + br) where
  features: [32768, 1024] f32
  W:        [64, 2, 1024] f32  (stacked per-branch Linear(L, 2) weights)
  b:        [64, 2] f32
returning (out[..., 0], out[..., 1]) as two [32768, 64] f32 arrays.

Strategy: data-parallel over batch across 8 NeuronCores (4096 rows each).
The TensorEngine contracts over the partition dim, so features are repacked
host-side into a transposed, tile-contiguous layout (free w.r.t. HW time).

The correctness gate is rel_l2 < 2e-2; plain fp16 x/w with f32 PSUM
accumulation and an fp16 output store lands at ~4e-4, so everything runs
single-term fp16.  Per-core HBM traffic is 8 MB x + 1 MB out + 0.25 MB w
(~9.25 MB -> ~26 us at 358 GB/s), and PE work is 64 matmuls x 512 cols
(~14 us at 2.4 GHz), leaving the kernel memory-bound on the x stream.
"""

import sys

for _p in ("/opt/trn_rl_repo", "/root/.axon_site"):
    if _p not in sys.path:
        sys.path.insert(0, _p)

import numpy as np

import concourse.mybir as mybir
import concourse.tile as tile
from concourse import bacc
from concourse.bass_utils import run_bass_kernel_spmd

# Problem shapes (hardcoded per contract)
B, L, A = 32768, 1024, 64
NCORES = 8
BS = B // NCORES          # 4096 batch rows per core
KO = L // 128             # 8 contraction slices
CH = 2 * A                # 128 output channels (c = k*64 + a)

F32 = mybir.dt.float32
F16 = mybir.dt.float16

# DMA chunk widths (batch columns per core) and ko-granularity per chunk.
# Big head chunks issue as ONE descriptor-generation job each with large
# per-partition-contiguous runs (32/24 KB), minimizing DGE serialization on
# the Sync sequencer; the small 512-wide tail chunk loads in ko-pairs so the
# final matmuls chase the stream instead of waiting on a 2 MB landing.
CHUNKS = [2048, 1536, 512]
CHUNK_HS = [8, 8, 2]
assert sum(CHUNKS) == BS
MM_N = 512  # moving free dim per matmul (fp16 cap / one fp32 PSUM bank)

_NC = None


def _build_nc():
    nc = bacc.Bacc()
    # x is packed chunk-major on the host: for each chunk (cn columns), the
    # per-partition bytes are one contiguous (ko, n) block of KO*cn elements.
    xh = nc.dram_tensor("xh", [128, KO * BS], F16, kind="ExternalInput")
    wh = nc.dram_tensor("wh", [128, KO, CH], F16, kind="ExternalInput")
    bvec = nc.dram_tensor("bias", [CH, 1], F32, kind="ExternalInput")
    out = nc.dram_tensor("out", [CH, BS], F16, kind="ExternalOutput")

    with tile.TileContext(nc) as tc:
        with (
            tc.tile_pool(name="consts", bufs=1) as consts,
            tc.tile_pool(name="xhp", bufs=1) as xhp,
            tc.tile_pool(name="op", bufs=4) as op,
            tc.tile_pool(name="ps", bufs=4, space="PSUM") as ps,
            tc.tile_pool(name="warm", bufs=1, space="PSUM") as warm_ps,
        ):
            # PE warmup: ~10 dependency-free matmuls on zeroed tiles fill the
            # otherwise-idle window while the first loads stream in, so the
            # HAM clock gate is already at 8/8 (2.4 GHz) when real matmuls
            # start (saves the ~2x-slow cold ramp on the critical path).
            w_warm = consts.tile([128, CH], F16)
            nc.vector.memset(w_warm[:], 0.0)
            x_warm = consts.tile([128, MM_N], F16)
            nc.gpsimd.memset(x_warm[:], 0.0)
            pw = warm_ps.tile([CH, MM_N], F32)
            for i in range(10):
                nc.tensor.matmul(
                    pw[:], w_warm[:], x_warm[:], start=(i == 0), stop=(i == 9)
                )
            # Ring assignment: the Sync (SP) HWDGE ring is purely the x
            # stream in need-order.  The Scalar (ACT) ring loads the small
            # constants up front (before any ACTIVATE exists, so no convoy),
            # then does activations + out-stores; a store depends on its own
            # activation, so no convoy can form there either.
            wh_sb = consts.tile([128, KO, CH], F16)
            nc.scalar.dma_start(wh_sb[:], wh[:])
            b_sb = consts.tile([CH, 1], F32)
            nc.scalar.dma_start(b_sb[:], bvec[:])

            # Issue ALL x loads up front on the Sync ring: every chunk has
            # its own SBUF tile (64 KB/partition total), so no load ever
            # waits on a tile release and the ring streams continuously at
            # HBM rate.
            xh_tiles = []
            n0 = 0
            for ci, cn in enumerate(CHUNKS):
                off = KO * n0
                src_h = xh[:, off : off + KO * cn].rearrange(
                    "p (ko n) -> p ko n", ko=KO
                )
                xh_sb = xhp.tile([128, KO, cn], F16, tag=f"xh{ci}", name="xh_sb")
                hs = CHUNK_HS[ci]
                for k0 in range(0, KO, hs):
                    nc.sync.dma_start(
                        xh_sb[:, k0 : k0 + hs], src_h[:, k0 : k0 + hs]
                    )
                xh_tiles.append(xh_sb)
                n0 += cn

            # Compute in 512-col slabs: one PSUM bank per slab, 8 ko matmuls
            # accumulating, then a fused bias+tanh activation straight to an
            # fp16 SBUF tile and a store on the ACT ring.
            n0 = 0
            for ci, cn in enumerate(CHUNKS):
                xh_sb = xh_tiles[ci]
                for s0 in range(0, cn, MM_N):
                    s1 = min(s0 + MM_N, cn)
                    sw = s1 - s0
                    pt = ps.tile([CH, MM_N], F32, tag="pt", name="pt")[:, :sw]
                    for ko in range(KO):
                        nc.tensor.matmul(
                            pt[:],
                            wh_sb[:, ko],
                            xh_sb[:, ko, s0:s1],
                            start=(ko == 0),
                            stop=(ko == KO - 1),
                        )
                    o_sb = op.tile([CH, MM_N], F16, tag="o", name="o_sb")[:, :sw]
                    nc.scalar.activation(
                        o_sb[:],
                        pt[:],
                        mybir.ActivationFunctionType.Tanh,
                        bias=b_sb[:, 0:1],
                        scale=1.0,
                    )
                    # Store via the ACT engine's HWDGE ring: the store depends
                    # on its own activation, so no convoy forms, and the Sync
                    # ring stays free to stream xh loads.
                    nc.scalar.dma_start(out[:, n0 + s0 : n0 + s1], o_sb[:])
                n0 += cn
    nc.compile()
    return nc


def _get_nc():
    global _NC
    if _NC is None:
        _NC = _build_nc()
    return _NC


def _pack_x(shard16):
    # shard16 [BS, L] -> chunk-major [128, KO*BS]: per partition p, chunk c
    # occupies a contiguous (ko, n) block.
    shT = shard16.T  # [L, BS] view
    parts = []
    n0 = 0
    for cn in CHUNKS:
        blk = (
            shT[:, n0 : n0 + cn]
            .reshape(KO, 128, cn)
            .transpose(1, 0, 2)
            .reshape(128, KO * cn)
        )
        parts.append(blk)
        n0 += cn
    return np.ascontiguousarray(np.concatenate(parts, axis=1))


def _shard_inputs(features, W, b):
    features = np.ascontiguousarray(features, dtype=np.float32)
    W = np.ascontiguousarray(W, dtype=np.float32)
    b = np.ascontiguousarray(b, dtype=np.float32)

    # Wr[l, c] with c = k*A + a; fp16, device layout [p, ko, c]
    wr = W.transpose(2, 1, 0).reshape(L, CH)
    wr_h = wr.astype(np.float16)
    wh_dev = np.ascontiguousarray(wr_h.reshape(KO, 128, CH).transpose(1, 0, 2))
    b_dev = np.ascontiguousarray(b.transpose(1, 0).reshape(CH, 1))

    in_maps = []
    for i in range(NCORES):
        sh = features[i * BS : (i + 1) * BS]  # [BS, L]
        sh_h = sh.astype(np.float16)
        in_maps.append(
            {
                "xh": _pack_x(sh_h),
                "wh": wh_dev,
                "bias": b_dev,
            }
        )
    return in_maps


def _gather(results):
    out0 = np.empty((B, A), dtype=np.float32)
    out1 = np.empty((B, A), dtype=np.float32)
    for i, r in enumerate(results):
        arr = r["out"].T.astype(np.float32)  # [CH, BS] -> [BS, CH]
        out0[i * BS : (i + 1) * BS] = arr[:, :A]
        out1[i * BS : (i + 1) * BS] = arr[:, A:]
    return out0, out1


def _run(inputs, trace=False, trace_cores=None):
    nc = _get_nc()
    in_maps = _shard_inputs(inputs["features"], inputs["W"], inputs["b"])
    res = run_bass_kernel_spmd(
        nc,
        in_maps,
        core_ids=list(range(NCORES)),
        trace=trace,
        trace_cores=trace_cores,
    )
    return _gather(res.results), res


def kernel(features, W, b):
    (out0, out1), _ = _run({"features": features, "W": W, "b": b})
    return out0, out1


# revision 6
# speedup vs baseline: 1.0676x; 1.0676x over previous
"""Bass/Trainium2 kernel for nn_BranchedPolicyNetwork.

Computes out = tanh(features @ Wr + br) where
  features: [32768, 1024] f32
  W:        [64, 2, 1024] f32  (stacked per-branch Linear(L, 2) weights)
  b:        [64, 2] f32
returning (out[..., 0], out[..., 1]) as two [32768, 64] f32 arrays.

Strategy: data-parallel over batch across 8 NeuronCores (4096 rows each).
The TensorEngine contracts over the partition dim, so features are repacked
host-side into a transposed, tile-contiguous layout (free w.r.t. HW time).

The correctness gate is rel_l2 < 2e-2; plain fp16 x/w with f32 PSUM
accumulation and an fp16 output store lands at ~4e-4, so everything runs
single-term fp16.  Per-core HBM traffic is 8 MB x + 1 MB out + 0.25 MB w
(~9.25 MB -> ~26 us at 358 GB/s), and PE work is 64 matmuls x 512 cols
(~14 us at 2.4 GHz), leaving the kernel memory-bound on the x stream.
"""

import sys

for _p in ("/opt/trn_rl_repo", "/root/.axon_site"):
    if _p not in sys.path:
        sys.path.insert(0, _p)

import numpy as np

import concourse.mybir as mybir
import concourse.tile as tile
from concourse import bacc
from concourse.bass_utils import run_bass_kernel_spmd

# Problem shapes (hardcoded per contract)
B, L, A = 32768, 1024, 64
NCORES = 8
BS = B // NCORES          # 4096 batch rows per core
KO = L // 128             # 8 contraction slices
CH = 2 * A                # 128 output channels (c = k*64 + a)

F32 = mybir.dt.float32
F16 = mybir.dt.float16

# DMA chunk widths (batch columns per core) and ko-granularity per chunk.
# Chunk = one 512-col matmul slab: each loads as ONE descriptor-generation
# job with 8KB per-partition-contiguous runs (vs 4KB at ko-pair granularity),
# and the dependency unit matches exactly what one slab's matmuls consume, so
# compute starts per-slab as the stream lands.  The final chunk loads in
# ko-pairs so only the last 2 of its 8 matmuls wait on the stream tail.
CHUNKS = [512] * 8
CHUNK_HS = [8] * 7 + [2]
assert sum(CHUNKS) == BS
MM_N = 512  # moving free dim per matmul (fp16 cap / one fp32 PSUM bank)

_NC = None


def _build_nc():
    nc = bacc.Bacc()
    # x is packed chunk-major on the host: for each chunk (cn columns), the
    # per-partition bytes are one contiguous (ko, n) block of KO*cn elements.
    xh = nc.dram_tensor("xh", [128, KO * BS], F16, kind="ExternalInput")
    wh = nc.dram_tensor("wh", [128, KO, CH], F16, kind="ExternalInput")
    bvec = nc.dram_tensor("bias", [CH, 1], F32, kind="ExternalInput")
    out = nc.dram_tensor("out", [CH, BS], F16, kind="ExternalOutput")

    with tile.TileContext(nc) as tc:
        with (
            tc.tile_pool(name="consts", bufs=1) as consts,
            tc.tile_pool(name="xhp", bufs=1) as xhp,
            tc.tile_pool(name="op", bufs=4) as op,
            tc.tile_pool(name="ps", bufs=4, space="PSUM") as ps,
            tc.tile_pool(name="warm", bufs=1, space="PSUM") as warm_ps,
        ):
            # PE warmup: ~10 dependency-free matmuls on zeroed tiles fill the
            # otherwise-idle window while the first loads stream in, so the
            # HAM clock gate is already at 8/8 (2.4 GHz) when real matmuls
            # start (saves the ~2x-slow cold ramp on the critical path).
            w_warm = consts.tile([128, CH], F16)
            nc.vector.memset(w_warm[:], 0.0)
            x_warm = consts.tile([128, MM_N], F16)
            nc.gpsimd.memset(x_warm[:], 0.0)
            pw = warm_ps.tile([CH, MM_N], F32)
            for i in range(10):
                nc.tensor.matmul(
                    pw[:], w_warm[:], x_warm[:], start=(i == 0), stop=(i == 9)
                )
            # Ring assignment: the Sync (SP) HWDGE ring is purely the x
            # stream in need-order.  The Scalar (ACT) ring loads the small
            # constants up front (before any ACTIVATE exists, so no convoy),
            # then does activations + out-stores; a store depends on its own
            # activation, so no convoy can form there either.
            wh_sb = consts.tile([128, KO, CH], F16)
            nc.scalar.dma_start(wh_sb[:], wh[:])
            b_sb = consts.tile([CH, 1], F32)
            nc.scalar.dma_start(b_sb[:], bvec[:])

            # Issue ALL x loads up front on the Sync ring: every chunk has
            # its own SBUF tile (64 KB/partition total), so no load ever
            # waits on a tile release and the ring streams continuously at
            # HBM rate.
            xh_tiles = []
            n0 = 0
            for ci, cn in enumerate(CHUNKS):
                off = KO * n0
                src_h = xh[:, off : off + KO * cn].rearrange(
                    "p (ko n) -> p ko n", ko=KO
                )
                xh_sb = xhp.tile([128, KO, cn], F16, tag=f"xh{ci}", name="xh_sb")
                hs = CHUNK_HS[ci]
                for k0 in range(0, KO, hs):
                    nc.sync.dma_start(
                        xh_sb[:, k0 : k0 + hs], src_h[:, k0 : k0 + hs]
                    )
                xh_tiles.append(xh_sb)
                n0 += cn

            # Compute in 512-col slabs: one PSUM bank per slab, 8 ko matmuls
            # accumulating, then a fused bias+tanh activation straight to an
            # fp16 SBUF tile and a store on the ACT ring.
            n0 = 0
            for ci, cn in enumerate(CHUNKS):
                xh_sb = xh_tiles[ci]
                for s0 in range(0, cn, MM_N):
                    s1 = min(s0 + MM_N, cn)
                    sw = s1 - s0
                    pt = ps.tile([CH, MM_N], F32, tag="pt", name="pt")[:, :sw]
                    for ko in range(KO):
                        nc.tensor.matmul(
                            pt[:],
                            wh_sb[:, ko],
                            xh_sb[:, ko, s0:s1],
                            start=(ko == 0),
                            stop=(ko == KO - 1),
                        )
                    o_sb = op.tile([CH, MM_N], F16, tag="o", name="o_sb")[:, :sw]
                    nc.scalar.activation(
                        o_sb[:],
                        pt[:],
                        mybir.ActivationFunctionType.Tanh,
                        bias=b_sb[:, 0:1],
                        scale=1.0,
                    )
                    # Store via the ACT engine's HWDGE ring: the store depends
                    # on its own activation, so no convoy forms, and the Sync
                    # ring stays free to stream xh loads.
                    nc.scalar.dma_start(out[:, n0 + s0 : n0 + s1], o_sb[:])
                n0 += cn
    nc.compile()
    return nc


def _get_nc():
    global _NC
    if _NC is None:
        _NC = _build_nc()
    return _NC


def _pack_x(shard16):
    # shard16 [BS, L] -> chunk-major [128, KO*BS]: per partition p, chunk c
    # occupies a contiguous (ko, n) block.
    shT = shard16.T  # [L, BS] view
    parts = []
    n0 = 0
    for cn in CHUNKS:
        blk = (
            shT[:, n0 : n0 + cn]
            .reshape(KO, 128, cn)
            .transpose(1, 0, 2)
            .reshape(128, KO * cn)
        )
        parts.append(blk)
        n0 += cn
    return np.ascontiguousarray(np.concatenate(parts, axis=1))


def _shard_inputs(features, W, b):
    features = np.ascontiguousarray(features, dtype=np.float32)
    W = np.ascontiguousarray(W, dtype=np.float32)
    b = np.ascontiguousarray(b, dtype=np.float32)

    # Wr[l, c] with c = k*A + a; fp16, device layout [p, ko, c]
    wr = W.transpose(2, 1, 0).reshape(L, CH)
    wr_h = wr.astype(np.float16)
    wh_dev = np.ascontiguousarray(wr_h.reshape(KO, 128, CH).transpose(1, 0, 2))
    b_dev = np.ascontiguousarray(b.transpose(1, 0).reshape(CH, 1))

    in_maps = []
    for i in range(NCORES):
        sh = features[i * BS : (i + 1) * BS]  # [BS, L]
        sh_h = sh.astype(np.float16)
        in_maps.append(
            {
                "xh": _pack_x(sh_h),
                "wh": wh_dev,
                "bias": b_dev,
            }
        )
    return in_maps


def _gather(results):
    out0 = np.empty((B, A), dtype=np.float32)
    out1 = np.empty((B, A), dtype=np.float32)
    for i, r in enumerate(results):
        arr = r["out"].T.astype(np.float32)  # [CH, BS] -> [BS, CH]
        out0[i * BS : (i + 1) * BS] = arr[:, :A]
        out1[i * BS : (i + 1) * BS] = arr[:, A:]
    return out0, out1


def _run(inputs, trace=False, trace_cores=None):
    nc = _get_nc()
    in_maps = _shard_inputs(inputs["features"], inputs["W"], inputs["b"])
    res = run_bass_kernel_spmd(
        nc,
        in_maps,
        core_ids=list(range(NCORES)),
        trace=trace,
        trace_cores=trace_cores,
    )
    return _gather(res.results), res


def kernel(features, W, b):
    (out0, out1), _ = _run({"features": features, "W": W, "b": b})
    return out0, out1
